# revision 57
# baseline (speedup 1.0000x reference)
"""Trainium2 Bass kernel for nn_MultiHeadAttention_59227599012491.

Reference computation (per batch b):
    xf = x[b].reshape(S, 256)
    q  = softplus(xf @ Wq.T + bq);  k = softplus(xf @ Wk.T + bk)
    v  = xf @ Wv.T + bv
    weight = q @ k.T            (no softmax!)
    result = weight @ v
    out    = result @ Wo.T + bo

Because there is no softmax, attention is associative:
    result = (q @ k.T) @ v = q @ (k.T @ v) = q @ G,   G: [256, 256]
    out    = q @ (G @ Wo.T) + bo = q @ M + bo
so the S x S score matrix never needs to be materialized. Per-core work
drops to a handful of [*, 256] x [256, 256] matmuls; the kernel is
jointly engine-bound (PE ~32 us, ACT ~29 us, DVE ~28 us busy/core).

Hard-won scheduling facts (all measured): the three DMA-issuing rings
(sync/scalar HWDGE, gpsimd SWDGE) share the same 16 physical DMA
engines, so the single sync-ring FIFO *is* the priority mechanism --
spreading input loads across rings only slows the critical pieces.
GPSIMD cannot touch PSUM; two PSUM accumulation groups must not share
a bank; an NRT AllReduce has ~17 us fixed rendezvous latency. fp32
PSUM evictions are uop-pinned to 1x on the DVE (one PSUM read port)
and ScalarE is 1 elem/cycle for all dtypes, so the elementwise floors
are hard. DMA-order rule: FIFO position in the single sync ring is
priority; order each engine's gate tensor so all gates of the first
dependent op complete SIMULTANEOUSLY (biasc right after x0 balances
the DVE's bias gate against its matmul gate -- either extreme costs
1-3 us on the DVE-paced loop). Descriptor count = DRAM row count per
tensor; the head floor is 512 descriptors (~5.5 us) before the first
matmul.

Sharding: B=4 batches x 2 query-halves -> 8 cores, no collectives.
(An NRT AllReduce of the tiny M matrix was measured at ~17 us fixed
rendezvous latency on this runtime -- more than the k/v/G dedup saves --
so each core recomputes k/v/G/M for its whole batch and only the
query/output rows are split across the pair.)

Layouts (PE computes out = lhsT.T @ rhs, contracting partition dim):
    xbT  [256, 4096]  x[b] transposed on host (queries first SQ cols)
    qT   [256, 2048]  lhsT = WqT tile, rhs = xbT     (softplus via ACT,
                      bias per-partition, fused into the Exp pass)
    kv   [4096, 512]  k and v fused: rhs = [WkT | WvT], one stationary
                      xbT tile per row tile serves both. +[bk|bv] via a
                      single DVE add; softplus on the k half in-place
                      (ACT Exp then Ln(1+t); batch-size ramp 2,4,8,8
                      then 4,2,2,2 -- small first so the ~98%-saturated
                      ACT engine starts ASAP, small last so the final
                      softplus -> GT -> M -> out chain stays short)
    GT   [256, 256]   GT[d,e] = sum_s v[s,d] k[s,e]: lhsT = v t, rhs = k t
    M    [256, 256]   M[e,do] = sum_d GT[d,e] WoT[d,do]: lhsT = GT, rhs = WoT
    outT [256, 2048]  transposed output: lhsT = M block (stationary,
                      reused across s), rhs = qT chunk; bo is then
                      per-partition (DVE tensor_scalar_add) and the fp16
                      dump has 2 KB descriptor runs -- half the output
                      bytes and a quarter of the descriptors vs the
                      natural-layout fp32 store; host un-transposes

The tile scheduler interleaves the qT/GT/out matmuls into the DVE-paced
kv loop's PE gaps, so the PE runs at ~91% occupancy over its window;
the engines are jointly near-saturated (PE ~32 us, ACT ~29 us, DVE
~28 us busy per core) and the phase structure below measures faster
than every explicitly-interleaved variant tried.

The activation-table pass is steered to `natural_log_exp_and_others`
(the only set holding Exp AND Ln) so the ACT engine loads its PWP table
once instead of reloading per activation (24 loads ~= 30us saved).
"""

import numpy as np

S = 4096
SQ = 2048  # query rows per core
D = 256
P = 128
IT = D // P  # 2 input-dim tiles
DT = D // P  # 2 d-model tiles
NS = S // P  # 32 sequence tiles
BLK = 512  # free-dim block for qT
N_CORES = 8

MM_DTYPE_NAME = "float16"

_CACHE = {}


def _patched_act_tables(orig_fn):
    def patched(arch):
        tabs = orig_fn(arch)
        return {
            name: (s if name == "natural_log_exp_and_others" else set())
            for name, s in tabs.items()
        }

    return patched


def _build_nc():
    import concourse.bacc as bacc
    import concourse.mybir as mybir
    import concourse.tile as tile

    FP = mybir.dt.float32
    FR = getattr(mybir.dt, MM_DTYPE_NAME)
    AF = mybir.ActivationFunctionType
    ADD = mybir.AluOpType.add

    nc = bacc.Bacc("TRN2", target_bir_lowering=False, debug=False, num_devices=1)

    xbT_d = nc.declare_dram_parameter("xbT", [D, S], FR, isOutput=False)
    # weights host-packed with it-blocks side by side: 2 KB descriptor rows
    wkvp_d = nc.declare_dram_parameter("wkvp", [P, 1024], FR, isOutput=False)
    wqop_d = nc.declare_dram_parameter("wqop", [P, 1024], FR, isOutput=False)
    # all biases in one [128, 516] fp32 tensor (cols 0:4 = bqT|boT,
    # 4:516 = host-replicated [bk|bv] row): one early DMA, 2 KB rows,
    # so the DVE and ACT queue gates both clear right after wkv
    bias_d = nc.declare_dram_parameter("biasc", [P, 4 + 2 * D], FP, isOutput=False)
    outp_d = nc.declare_dram_parameter("outp", [P, 2 * SQ], FR, isOutput=True)

    def mm(psum, lhsT, rhs, start, stop):
        nc.tensor.matmul(psum, lhsT, rhs, start=start, stop=stop)

    with tile.TileContext(nc) as tc:
        with (
            tc.tile_pool(name="w", bufs=1) as wpool,
            tc.tile_pool(name="big", bufs=1) as big,
            tc.tile_pool(name="tmp", bufs=4) as tpool,
            tc.tile_pool(name="psQ", bufs=3, space="PSUM") as psQ,
            tc.tile_pool(name="psKV", bufs=3, space="PSUM") as psKV,
            tc.tile_pool(name="psG", bufs=2, space="PSUM") as psG,
        ):
            wkv_sb = wpool.tile([P, 2 * 512], FR, tag="wkv")
            wqo_sb = wpool.tile([P, 2 * 512], FR, tag="wqo")
            xbT_sb = big.tile([P, IT, S], FR, tag="xbT")
            biasc = wpool.tile([P, 4 + 2 * D], FP, tag="biasc")
            bias_sb = biasc[:, 0:4]
            bc_bkv = biasc[:, 4 : 4 + 2 * D]
            b_bc = {"bkv": bc_bkv}
            nc.sync.dma_start(wkv_sb[:, :], wkvp_d.ap()[:, :])
            for it in range(IT):
                nc.sync.dma_start(
                    xbT_sb[:, it, 0:1024], xbT_d.ap()[it * P : (it + 1) * P, 0:1024]
                )
            # biasc directly after x0: the DVE's two gates (bias DMA and
            # the first tiles' matmuls) then complete simultaneously --
            # before x0 it over-serves the bias gate and the first matmul
            # slips ~1 us; after x1 the bias gate dominates by ~3 us
            nc.sync.dma_start(biasc[:, :], bias_d.ap()[:, :])
            # wqo before the second x chunk: the qT matmuls are the PE's
            # early gap-filler and their LDWEIGHTS gate on this tensor
            nc.sync.dma_start(wqo_sb[:, :], wqop_d.ap()[:, :])
            for it in range(IT):
                nc.sync.dma_start(
                    xbT_sb[:, it, 1024:2048], xbT_d.ap()[it * P : (it + 1) * P, 1024:2048]
                )
            for it in range(IT):
                nc.sync.dma_start(
                    xbT_sb[:, it, 2048:3072], xbT_d.ap()[it * P : (it + 1) * P, 2048:3072]
                )
            for it in range(IT):
                nc.sync.dma_start(
                    xbT_sb[:, it, 3072:4096], xbT_d.ap()[it * P : (it + 1) * P, 3072:4096]
                )

            kv_sb = big.tile([P, 2, NS, D], FR, tag="kv")
            qT_sb = big.tile([P, DT, SQ], FR, tag="qT")
            outT_sb = big.tile([P, DT, SQ], FR, tag="outT")
            GT_sb = wpool.tile([P, DT, D], FR, tag="GT")
            M_sb = wpool.tile([P, DT, D], FR, tag="M")

            for t in range(NS):
                ts = slice(t * P, (t + 1) * P)
                ps = psKV.tile([P, 2 * D], FP, tag="psKV")
                for it in range(IT):
                    mm(ps[:, :], xbT_sb[:, it, ts], wkv_sb[:, it * 512 : (it + 1) * 512], it == 0, it == IT - 1)
                nc.vector.tensor_tensor(
                    kv_sb[:, :, t, :], ps[:, :].rearrange("p (j d) -> p j d", j=2),
                    b_bc["bkv"][:, :].rearrange("p (j d) -> p j d", j=2), op=ADD,
                )
                # batch-size ramp: 2,4,8,8,8 then 2 at the end -- early
                # batches small so the saturated ACT engine starts ASAP,
                # final batches small so the softplus->GT->M->out chain
                # after the last kv tile stays short
                SPB = {1: 2, 5: 4, 13: 8, 21: 8, 25: 4, 27: 2, 29: 2, 31: 2}
                bsz = SPB.get(t, 0)
                if bsz:
                    tt = slice(t - bsz + 1, t + 1)
                    tmp = tpool.tile([P, bsz, D], FP, tag=f"tmpk{bsz}")
                    nc.scalar.activation(tmp[:, :, :], kv_sb[:, 0, tt, :], AF.Exp)
                    nc.scalar.activation(kv_sb[:, 0, tt, :], tmp[:, :, :], AF.Ln, bias=1.0)

            for dt in range(DT):
                for half in range(SQ // (2 * BLK)):
                    tmp = tpool.tile([P, 2, BLK], FP, tag="tmpq")
                    for c in range(2):
                        blk = 2 * half + c
                        ss = slice(blk * BLK, (blk + 1) * BLK)
                        ps = psQ.tile([P, BLK], FP, tag="psQ")
                        for it in range(IT):
                            mm(ps[:, :], wqo_sb[:, it * D + dt * P : it * D + (dt + 1) * P], xbT_sb[:, it, ss], it == 0, it == IT - 1)
                        nc.scalar.activation(
                            tmp[:, c, :], ps[:, :], AF.Exp, bias=bias_sb[:, dt : dt + 1]
                        )
                    nc.scalar.activation(
                        qT_sb[:, dt, 2 * half * BLK : 2 * (half + 1) * BLK],
                        tmp[:, :, :].rearrange("p a b -> p (a b)"),
                        AF.Ln,
                        bias=1.0,
                    )

            for dt in range(DT):
                vs = slice(dt * P, (dt + 1) * P)
                ps = psG.tile([P, D], FP, tag="psG")
                for t in range(NS):
                    mm(ps[:, :], kv_sb[:, 1, t, vs], kv_sb[:, 0, t, :], t == 0, t == NS - 1)
                nc.vector.tensor_copy(GT_sb[:, dt, :], ps[:, :])

            for et in range(DT):
                es = slice(et * P, (et + 1) * P)
                ps = psG.tile([P, D], FP, tag="psG")
                for dt in range(DT):
                    mm(ps[:, :], GT_sb[:, dt, es], wqo_sb[:, 512 + dt * D : 512 + (dt + 1) * D], dt == 0, dt == DT - 1)
                nc.vector.tensor_copy(M_sb[:, et, :], ps[:, :])

            # outT[do, s] = M^T q^T + bo: lhsT = M block (stationary,
            # reused across s), per-partition bo bias on the DVE, fp16
            # transposed dump with 2 KB descriptor runs
            for dot in range(DT):
                for blk in range(SQ // BLK):
                    ss = slice(blk * BLK, (blk + 1) * BLK)
                    ps = psQ.tile([P, BLK], FP, tag="psQ")
                    for et in range(DT):
                        mm(
                            ps[:, :],
                            M_sb[:, et, dot * P : (dot + 1) * P],
                            qT_sb[:, et, ss],
                            et == 0,
                            et == DT - 1,
                        )
                    if dot == DT - 1:
                        # ACT is drained by now; Identity(in + bo) keeps the
                        # final eviction off the backlogged DVE queue
                        nc.scalar.activation(
                            outT_sb[:, dot, ss],
                            ps[:, :],
                            AF.Identity,
                            bias=bias_sb[:, 2 + dot : 3 + dot],
                        )
                    else:
                        nc.vector.tensor_scalar_add(
                            outT_sb[:, dot, ss], ps[:, :], bias_sb[:, 2 + dot : 3 + dot]
                        )
                    if dot == DT - 1 and blk >= SQ // BLK - 2:
                        # ship the last two chunks individually so only one
                        # chunk's descriptors remain after the final eviction
                        off = dot * SQ + blk * BLK
                        src_ap = outT_sb[:, dot, blk * BLK : (blk + 1) * BLK]
                        if blk == SQ // BLK - 1:
                            nc.sync.dma_start(
                                outp_d.ap()[0:64, off : off + BLK], src_ap[0:64, :]
                            )
                            nc.scalar.dma_start(
                                outp_d.ap()[64:P, off : off + BLK], src_ap[64:P, :]
                            )
                        else:
                            nc.sync.dma_start(outp_d.ap()[:, off : off + BLK], src_ap)
                    elif blk % 2 == 1:
                        off = dot * SQ + (blk - 1) * BLK
                        src_ap = outT_sb[:, dot, (blk - 1) * BLK : (blk + 1) * BLK]
                        nc.sync.dma_start(
                            outp_d.ap()[:, off : off + 2 * BLK], src_ap
                        )

    import concourse.hw_specs as hw_specs

    orig = bacc.get_activation_tables
    bacc.get_activation_tables = _patched_act_tables(hw_specs.get_activation_tables)
    try:
        nc.compile()
    finally:
        bacc.get_activation_tables = orig
    return nc


def _get_nc():
    nc = _CACHE.get("nc")
    if nc is None:
        nc = _build_nc()
        _CACHE["nc"] = nc
    return nc


def make_in_maps(x, Wq, bq, Wk, bk, Wv, bv, Wo, bo):
    B = x.shape[0]
    mmnp = np.float16
    xf = np.asarray(x, dtype=np.float32).reshape(B, S, D)
    xfT = np.ascontiguousarray(xf.transpose(0, 2, 1).astype(mmnp))
    wkv2 = np.hstack([np.asarray(Wk, mmnp).T, np.asarray(Wv, mmnp).T])  # [256, 512]
    wkvp = np.ascontiguousarray(
        wkv2.reshape(2, P, 512).transpose(1, 0, 2).reshape(P, 1024)
    )
    wq2 = np.asarray(Wq, mmnp).T
    wo2 = np.asarray(Wo, mmnp).T
    wqop = np.ascontiguousarray(
        np.hstack([wq2[0:P], wq2[P:D], wo2[0:P], wo2[P:D]])
    )  # [128, (it0 wq|it1 wq|dt0 wo|dt1 wo)]
    bkvrow = np.concatenate([np.asarray(bk, np.float32), np.asarray(bv, np.float32)])
    biasc = np.ascontiguousarray(
        np.hstack(
            [
                np.stack(
                    [
                        np.asarray(bq, np.float32)[0:P],
                        np.asarray(bq, np.float32)[P:D],
                        np.asarray(bo, np.float32)[0:P],
                        np.asarray(bo, np.float32)[P:D],
                    ],
                    axis=1,
                ),
                np.tile(bkvrow, (P, 1)),
            ]
        )
    )
    shared = {
        "wkvp": wkvp,
        "wqop": wqop,
        "biasc": biasc,
    }
    in_maps = []
    for c in range(N_CORES):
        b, h = divmod(c, 2)
        xT = xfT[b]
        if h == 1:
            xT = np.concatenate([xT[:, SQ:], xT[:, :SQ]], axis=1)
        in_maps.append({"xbT": np.ascontiguousarray(xT), **shared})
    return in_maps


def assemble_out(results, x_shape):
    B, S_, H, W = x_shape
    out = np.empty((B, S_, D), np.float32)
    for c in range(N_CORES):
        b, h = divmod(c, 2)
        outp = results[c]["outp"]  # [128, 2*SQ] fp16: [p, dot*SQ + s]
        v = outp.reshape(P, DT, SQ).astype(np.float32)
        out[b, h * SQ : (h + 1) * SQ] = v.transpose(2, 1, 0).reshape(SQ, D)
    return out.reshape(B, S_, H, W)


def kernel(x, Wq, bq, Wk, bk, Wv, bv, Wo, bo, _trace=False):
    from concourse.bass_utils import run_bass_kernel_spmd

    nc = _get_nc()
    in_maps = make_in_maps(x, Wq, bq, Wk, bk, Wv, bv, Wo, bo)
    res = run_bass_kernel_spmd(nc, in_maps, list(range(N_CORES)), trace=_trace)
    out = assemble_out(res.results, x.shape)
    if _trace:
        _CACHE["last_result"] = res
    return out


# revision 58
# speedup vs baseline: 1.0410x; 1.0410x over previous
"""Trainium2 Bass kernel for nn_MultiHeadAttention_59227599012491.

Reference computation (per batch b):
    xf = x[b].reshape(S, 256)
    q  = softplus(xf @ Wq.T + bq);  k = softplus(xf @ Wk.T + bk)
    v  = xf @ Wv.T + bv
    weight = q @ k.T            (no softmax!)
    result = weight @ v
    out    = result @ Wo.T + bo

Because there is no softmax, attention is associative:
    result = (q @ k.T) @ v = q @ (k.T @ v) = q @ G,   G: [256, 256]
    out    = q @ (G @ Wo.T) + bo = q @ M + bo
so the S x S score matrix never needs to be materialized. Per-core work
drops to a handful of [*, 256] x [256, 256] matmuls; the kernel is
jointly engine-bound (PE ~32 us, ACT ~29 us, DVE ~28 us busy/core).

Hard-won scheduling facts (all measured): the three DMA-issuing rings
(sync/scalar HWDGE, gpsimd SWDGE) share the same 16 physical DMA
engines, so the single sync-ring FIFO *is* the priority mechanism --
spreading input loads across rings only slows the critical pieces.
GPSIMD cannot touch PSUM; two PSUM accumulation groups must not share
a bank; an NRT AllReduce has ~17 us fixed rendezvous latency. fp32
PSUM evictions are uop-pinned to 1x on the DVE (one PSUM read port)
and ScalarE is 1 elem/cycle for all dtypes, so the elementwise floors
are hard. DMA-order rule: FIFO position in the single sync ring is
priority; order each engine's gate tensor so all gates of the first
dependent op complete SIMULTANEOUSLY (biasc right after x0 balances
the DVE's bias gate against its matmul gate -- either extreme costs
1-3 us on the DVE-paced loop). Descriptor count = DRAM row count per
tensor; the head floor is 512 descriptors (~5.5 us) before the first
matmul.

Sharding: B=4 batches x 2 query-halves -> 8 cores, no collectives.
(An NRT AllReduce of the tiny M matrix was measured at ~17 us fixed
rendezvous latency on this runtime -- more than the k/v/G dedup saves --
so each core recomputes k/v/G/M for its whole batch and only the
query/output rows are split across the pair.)

Layouts (PE computes out = lhsT.T @ rhs, contracting partition dim):
    xbT  [256, 4096]  x[b] transposed on host (queries first SQ cols)
    qT   [256, 2048]  lhsT = WqT tile, rhs = xbT     (softplus via ACT,
                      bias per-partition, fused into the Exp pass)
    kv   [4096, 512]  k and v fused: rhs = [WkT | WvT], one stationary
                      xbT tile per row tile serves both. +[bk|bv] via a
                      single DVE add; softplus on the k half in-place
                      (ACT Exp then Ln(1+t); batch-size ramp 2,4,8,8
                      then 4,2,2,2 -- small first so the ~98%-saturated
                      ACT engine starts ASAP, small last so the final
                      softplus -> GT -> M -> out chain stays short)
    GT   [256, 256]   GT[d,e] = sum_s v[s,d] k[s,e]: lhsT = v t, rhs = k t
    M    [256, 256]   M[e,do] = sum_d GT[d,e] WoT[d,do]: lhsT = GT, rhs = WoT
    outT [256, 2048]  transposed output: lhsT = M block (stationary,
                      reused across s), rhs = qT chunk; bo is then
                      per-partition (DVE tensor_scalar_add) and the fp16
                      dump has 2 KB descriptor runs -- half the output
                      bytes and a quarter of the descriptors vs the
                      natural-layout fp32 store; host un-transposes

The tile scheduler interleaves the qT/GT/out matmuls into the DVE-paced
kv loop's PE gaps, so the PE runs at ~91% occupancy over its window;
the engines are jointly near-saturated (PE ~32 us, ACT ~29 us, DVE
~28 us busy per core) and the phase structure below measures faster
than every explicitly-interleaved variant tried.

The activation-table pass is steered to `natural_log_exp_and_others`
(the only set holding Exp AND Ln) so the ACT engine loads its PWP table
once instead of reloading per activation (24 loads ~= 30us saved).
"""

import numpy as np

S = 4096
SQ = 2048  # query rows per core
D = 256
P = 128
IT = D // P  # 2 input-dim tiles
DT = D // P  # 2 d-model tiles
NS = S // P  # 32 sequence tiles
BLK = 512  # free-dim block for qT
N_CORES = 8

MM_DTYPE_NAME = "float16"

_CACHE = {}


def _patched_act_tables(orig_fn):
    def patched(arch):
        tabs = orig_fn(arch)
        return {
            name: (s if name == "natural_log_exp_and_others" else set())
            for name, s in tabs.items()
        }

    return patched


def _build_nc():
    import concourse.bacc as bacc
    import concourse.mybir as mybir
    import concourse.tile as tile

    FP = mybir.dt.float32
    FR = getattr(mybir.dt, MM_DTYPE_NAME)
    AF = mybir.ActivationFunctionType
    ADD = mybir.AluOpType.add

    nc = bacc.Bacc("TRN2", target_bir_lowering=False, debug=False, num_devices=1)

    xbT_d = nc.declare_dram_parameter("xbT", [D, S], FR, isOutput=False)
    # weights host-packed with it-blocks side by side: 2 KB descriptor rows
    wkvp_d = nc.declare_dram_parameter("wkvp", [P, 1024], FR, isOutput=False)
    wqop_d = nc.declare_dram_parameter("wqop", [P, 1024], FR, isOutput=False)
    # all biases in one [128, 516] fp32 tensor (cols 0:4 = bqT|boT,
    # 4:516 = host-replicated [bk|bv] row): one early DMA, 2 KB rows,
    # so the DVE and ACT queue gates both clear right after wkv
    bias_d = nc.declare_dram_parameter("biasc", [P, 4 + 2 * D], FP, isOutput=False)
    outp_d = nc.declare_dram_parameter("outp", [P, 2 * SQ], FR, isOutput=True)

    def mm(psum, lhsT, rhs, start, stop):
        nc.tensor.matmul(psum, lhsT, rhs, start=start, stop=stop)

    with tile.TileContext(nc) as tc:
        with (
            tc.tile_pool(name="w", bufs=1) as wpool,
            tc.tile_pool(name="big", bufs=1) as big,
            tc.tile_pool(name="tmp", bufs=4) as tpool,
            tc.tile_pool(name="psQ", bufs=3, space="PSUM") as psQ,
            tc.tile_pool(name="psKV", bufs=3, space="PSUM") as psKV,
            tc.tile_pool(name="psG", bufs=2, space="PSUM") as psG,
        ):
            wkv_sb = wpool.tile([P, 2 * 512], FR, tag="wkv")
            wqo_sb = wpool.tile([P, 2 * 512], FR, tag="wqo")
            xbT_sb = big.tile([P, IT, S], FR, tag="xbT")
            biasc = wpool.tile([P, 4 + 2 * D], FP, tag="biasc")
            bias_sb = biasc[:, 0:4]
            bc_bkv = biasc[:, 4 : 4 + 2 * D]
            b_bc = {"bkv": bc_bkv}
            nc.sync.dma_start(wkv_sb[:, :], wkvp_d.ap()[:, :])
            for it in range(IT):
                nc.sync.dma_start(
                    xbT_sb[:, it, 0:1024], xbT_d.ap()[it * P : (it + 1) * P, 0:1024]
                )
            # biasc directly after x0: the DVE's two gates (bias DMA and
            # the first tiles' matmuls) then complete simultaneously --
            # before x0 it over-serves the bias gate and the first matmul
            # slips ~1 us; after x1 the bias gate dominates by ~3 us
            nc.sync.dma_start(biasc[:, :], bias_d.ap()[:, :])
            # wqo before the second x chunk: the qT matmuls are the PE's
            # early gap-filler and their LDWEIGHTS gate on this tensor
            nc.sync.dma_start(wqo_sb[:, :], wqop_d.ap()[:, :])
            for it in range(IT):
                nc.sync.dma_start(
                    xbT_sb[:, it, 1024:2048], xbT_d.ap()[it * P : (it + 1) * P, 1024:2048]
                )
            for it in range(IT):
                nc.sync.dma_start(
                    xbT_sb[:, it, 2048:3072], xbT_d.ap()[it * P : (it + 1) * P, 2048:3072]
                )
            for it in range(IT):
                nc.sync.dma_start(
                    xbT_sb[:, it, 3072:4096], xbT_d.ap()[it * P : (it + 1) * P, 3072:4096]
                )

            kv_sb = big.tile([P, 2, NS, D], FR, tag="kv")
            qT_sb = big.tile([P, DT, SQ], FR, tag="qT")
            outT_sb = big.tile([P, DT, SQ], FR, tag="outT")
            GT_sb = wpool.tile([P, DT, D], FR, tag="GT")
            M_sb = wpool.tile([P, DT, D], FR, tag="M")

            for t in range(NS):
                ts = slice(t * P, (t + 1) * P)
                ps = psKV.tile([P, 2 * D], FP, tag="psKV")
                for it in range(IT):
                    mm(ps[:, :], xbT_sb[:, it, ts], wkv_sb[:, it * 512 : (it + 1) * 512], it == 0, it == IT - 1)
                nc.vector.tensor_tensor(
                    kv_sb[:, :, t, :], ps[:, :].rearrange("p (j d) -> p j d", j=2),
                    b_bc["bkv"][:, :].rearrange("p (j d) -> p j d", j=2), op=ADD,
                )
                # batch-size ramp: 2,4,8,8,8 then 2 at the end -- early
                # batches small so the saturated ACT engine starts ASAP,
                # final batches small so the softplus->GT->M->out chain
                # after the last kv tile stays short
                SPB = {1: 2, 5: 4, 13: 8, 21: 8, 25: 4, 27: 2, 29: 2, 31: 2}
                bsz = SPB.get(t, 0)
                if bsz:
                    tt = slice(t - bsz + 1, t + 1)
                    tmp = tpool.tile([P, bsz, D], FP, tag=f"tmpk{bsz}")
                    nc.scalar.activation(tmp[:, :, :], kv_sb[:, 0, tt, :], AF.Exp)
                    nc.scalar.activation(kv_sb[:, 0, tt, :], tmp[:, :, :], AF.Ln, bias=1.0)

            for dt in range(DT):
                for half in range(SQ // (2 * BLK)):
                    tmp = tpool.tile([P, 2, BLK], FP, tag="tmpq")
                    for c in range(2):
                        blk = 2 * half + c
                        ss = slice(blk * BLK, (blk + 1) * BLK)
                        ps = psQ.tile([P, BLK], FP, tag="psQ")
                        for it in range(IT):
                            mm(ps[:, :], wqo_sb[:, it * D + dt * P : it * D + (dt + 1) * P], xbT_sb[:, it, ss], it == 0, it == IT - 1)
                        nc.scalar.activation(
                            tmp[:, c, :], ps[:, :], AF.Exp, bias=bias_sb[:, dt : dt + 1]
                        )
                    nc.scalar.activation(
                        qT_sb[:, dt, 2 * half * BLK : 2 * (half + 1) * BLK],
                        tmp[:, :, :].rearrange("p a b -> p (a b)"),
                        AF.Ln,
                        bias=1.0,
                    )

            for dt in range(DT):
                vs = slice(dt * P, (dt + 1) * P)
                ps = psG.tile([P, D], FP, tag="psG")
                for t in range(NS):
                    mm(ps[:, :], kv_sb[:, 1, t, vs], kv_sb[:, 0, t, :], t == 0, t == NS - 1)
                nc.vector.tensor_copy(GT_sb[:, dt, :], ps[:, :])

            for et in range(DT):
                es = slice(et * P, (et + 1) * P)
                ps = psG.tile([P, D], FP, tag="psG")
                for dt in range(DT):
                    mm(ps[:, :], GT_sb[:, dt, es], wqo_sb[:, 512 + dt * D : 512 + (dt + 1) * D], dt == 0, dt == DT - 1)
                nc.vector.tensor_copy(M_sb[:, et, :], ps[:, :])

            # outT[do, s] = M^T q^T + bo: lhsT = M block (stationary,
            # reused across s), per-partition bo bias on the DVE, fp16
            # transposed dump with 2 KB descriptor runs
            for dot in range(DT):
                for blk in range(SQ // BLK):
                    ss = slice(blk * BLK, (blk + 1) * BLK)
                    ps = psQ.tile([P, BLK], FP, tag="psQ")
                    for et in range(DT):
                        mm(
                            ps[:, :],
                            M_sb[:, et, dot * P : (dot + 1) * P],
                            qT_sb[:, et, ss],
                            et == 0,
                            et == DT - 1,
                        )
                    if dot == DT - 1 and blk == SQ // BLK - 1:
                        # very last chunk: split the eviction across the idle
                        # ACT and DVE so the serial tail halves
                        nc.scalar.activation(
                            outT_sb[:, dot, blk * BLK : blk * BLK + 256],
                            ps[:, 0:256],
                            AF.Identity,
                            bias=bias_sb[:, 2 + dot : 3 + dot],
                        )
                        nc.vector.tensor_scalar_add(
                            outT_sb[:, dot, blk * BLK + 256 : (blk + 1) * BLK],
                            ps[:, 256:512],
                            bias_sb[:, 2 + dot : 3 + dot],
                        )
                    elif dot == DT - 1:
                        # ACT is drained by now; Identity(in + bo) keeps the
                        # final eviction off the backlogged DVE queue
                        nc.scalar.activation(
                            outT_sb[:, dot, ss],
                            ps[:, :],
                            AF.Identity,
                            bias=bias_sb[:, 2 + dot : 3 + dot],
                        )
                    else:
                        nc.vector.tensor_scalar_add(
                            outT_sb[:, dot, ss], ps[:, :], bias_sb[:, 2 + dot : 3 + dot]
                        )
                    if dot == DT - 1 and blk >= SQ // BLK - 2:
                        # ship the last two chunks individually so only one
                        # chunk's descriptors remain after the final eviction
                        off = dot * SQ + blk * BLK
                        src_ap = outT_sb[:, dot, blk * BLK : (blk + 1) * BLK]
                        if blk == SQ // BLK - 1:
                            nc.sync.dma_start(
                                outp_d.ap()[0:64, off : off + BLK], src_ap[0:64, :]
                            )
                            nc.scalar.dma_start(
                                outp_d.ap()[64:P, off : off + BLK], src_ap[64:P, :]
                            )
                        else:
                            nc.sync.dma_start(outp_d.ap()[:, off : off + BLK], src_ap)
                    elif blk % 2 == 1:
                        off = dot * SQ + (blk - 1) * BLK
                        src_ap = outT_sb[:, dot, (blk - 1) * BLK : (blk + 1) * BLK]
                        nc.sync.dma_start(
                            outp_d.ap()[:, off : off + 2 * BLK], src_ap
                        )

    import concourse.hw_specs as hw_specs

    orig = bacc.get_activation_tables
    bacc.get_activation_tables = _patched_act_tables(hw_specs.get_activation_tables)
    try:
        nc.compile()
    finally:
        bacc.get_activation_tables = orig
    return nc


def _get_nc():
    nc = _CACHE.get("nc")
    if nc is None:
        nc = _build_nc()
        _CACHE["nc"] = nc
    return nc


def make_in_maps(x, Wq, bq, Wk, bk, Wv, bv, Wo, bo):
    B = x.shape[0]
    mmnp = np.float16
    xf = np.asarray(x, dtype=np.float32).reshape(B, S, D)
    xfT = np.ascontiguousarray(xf.transpose(0, 2, 1).astype(mmnp))
    wkv2 = np.hstack([np.asarray(Wk, mmnp).T, np.asarray(Wv, mmnp).T])  # [256, 512]
    wkvp = np.ascontiguousarray(
        wkv2.reshape(2, P, 512).transpose(1, 0, 2).reshape(P, 1024)
    )
    wq2 = np.asarray(Wq, mmnp).T
    wo2 = np.asarray(Wo, mmnp).T
    wqop = np.ascontiguousarray(
        np.hstack([wq2[0:P], wq2[P:D], wo2[0:P], wo2[P:D]])
    )  # [128, (it0 wq|it1 wq|dt0 wo|dt1 wo)]
    bkvrow = np.concatenate([np.asarray(bk, np.float32), np.asarray(bv, np.float32)])
    biasc = np.ascontiguousarray(
        np.hstack(
            [
                np.stack(
                    [
                        np.asarray(bq, np.float32)[0:P],
                        np.asarray(bq, np.float32)[P:D],
                        np.asarray(bo, np.float32)[0:P],
                        np.asarray(bo, np.float32)[P:D],
                    ],
                    axis=1,
                ),
                np.tile(bkvrow, (P, 1)),
            ]
        )
    )
    shared = {
        "wkvp": wkvp,
        "wqop": wqop,
        "biasc": biasc,
    }
    in_maps = []
    for c in range(N_CORES):
        b, h = divmod(c, 2)
        xT = xfT[b]
        if h == 1:
            xT = np.concatenate([xT[:, SQ:], xT[:, :SQ]], axis=1)
        in_maps.append({"xbT": np.ascontiguousarray(xT), **shared})
    return in_maps


def assemble_out(results, x_shape):
    B, S_, H, W = x_shape
    out = np.empty((B, S_, D), np.float32)
    for c in range(N_CORES):
        b, h = divmod(c, 2)
        outp = results[c]["outp"]  # [128, 2*SQ] fp16: [p, dot*SQ + s]
        v = outp.reshape(P, DT, SQ).astype(np.float32)
        out[b, h * SQ : (h + 1) * SQ] = v.transpose(2, 1, 0).reshape(SQ, D)
    return out.reshape(B, S_, H, W)


def kernel(x, Wq, bq, Wk, bk, Wv, bv, Wo, bo, _trace=False):
    from concourse.bass_utils import run_bass_kernel_spmd

    nc = _get_nc()
    in_maps = make_in_maps(x, Wq, bq, Wk, bk, Wv, bv, Wo, bo)
    res = run_bass_kernel_spmd(nc, in_maps, list(range(N_CORES)), trace=_trace)
    out = assemble_out(res.results, x.shape)
    if _trace:
        _CACHE["last_result"] = res
    return out
